# revision 9
# baseline (speedup 1.0000x reference)
"""Trainium2 Bass kernel for CenterWoParamMultiCosineNearLoss.

loss = mean_b [ S_b - m_b + (2*m_b^2 - Q_b) / S_b ]   where, per sample b,
  d_k = 1 - <x_b, c_{label_b, k}>  (k = 0..15 sub-centers of own class)
  S = sum_k d_k, Q = sum_k d_k^2, m = min_k d_k
(algebraically identical to the reference's term1+term2).

Layout strategy (v3): rows are grouped by CLASS on the host and packed into
uniform regions of G=128 columns, one class per region (classes larger than
G split; short regions zero-padded). Every core gets NREG regions, so the
SPMD program is uniform while the per-core class structure lives entirely
in the DMA'd data. The device needs NO per-row selection: each region's
DoubleRow fp8 matmul against its class's 16 sub-centers (lhsT [128,2,16])
streams the region's x columns and lands s = <x,c> directly as a [16, G]
PSUM stripe -- the 16 values per row ARE the row's own-class cosines.

Four regions stack into one [128, G] PSUM tile at partition offsets
0/32/64/96 (PE tile_position col offsets), so ONE DVE stream-transpose
(32x32 blocks) per group moves PSUM->SBUF AND transposes: each row's 16
values become contiguous along the free axis (the odd 16-wide slots are
zeros). Batched sum/sumsq/max reduces + the rowloss epilogue follow; the
all-zero slots produce exactly 14.125 each, subtracted in closed form on
the host together with the zero-padded columns.

fp8: x and centers are scaled by 16 each (keeps e4m3 in normal range); the
256x factor folds into the epilogue's existing affine constants. This
halves DMA vs fp16 and enables the DoubleRow matmul perf mode.
"""

import os
import sys

import numpy as np

for _p in ("/opt/trn_rl_repo", "/root/.axon_site/_ro/trn_rl_repo"):
    if os.path.isdir(_p) and _p not in sys.path:
        sys.path.append(_p)

import ml_dtypes  # noqa: E402

import concourse.tile as tile  # noqa: E402
from concourse import bacc  # noqa: E402
from concourse import mybir  # noqa: E402
from concourse.bass_utils import run_bass_kernel_spmd  # noqa: E402

P = 128          # SBUF partitions
B = 8192         # batch
D = 1024         # feature dim
K = 16           # sub-centers per class
NCORES = 8
KT2 = D // 256   # 4 DoubleRow contraction pairs (2 k-tiles each)
G = 128          # region width (columns per class-region; 32-multiple)
SCALE = 16.0     # per-tensor fp8 scale; total on s is SCALE*SCALE
PAD_LOSS = 14.125  # rowloss of an all-zero entry: 16 - 1 + (2-16)/16

_F32 = mybir.dt.float32
_F8 = mybir.dt.float8e4

_ADD = mybir.AluOpType.add
_MULT = mybir.AluOpType.mult
_SUB = mybir.AluOpType.subtract
_MAX = mybir.AluOpType.max
_AX = mybir.AxisListType.X
_DR = mybir.MatmulPerfMode.DoubleRow
_ACT = mybir.ActivationFunctionType


def _build_program(cfg):
    """One SPMD program for all 8 cores."""
    nreg = cfg[0]
    nchunk = nreg // 4            # DMA chunks (4 regions -> 4KB runs)
    ck = 4 * G                    # columns per DMA chunk
    wc = K * nreg                 # centers columns

    nc = bacc.Bacc(None, target_bir_lowering=False)
    xT = nc.declare_dram_parameter("xT", [nchunk, P, KT2, 2, ck], _F8, isOutput=False)
    cw = nc.declare_dram_parameter("cw", [P, KT2, 2, wc], _F8, isOutput=False)
    out = nc.declare_dram_parameter("out", [1, 1], _F32, isOutput=True)

    with tile.TileContext(nc) as tc:
        with (
            tc.tile_pool(name="const", bufs=1) as const,
            tc.tile_pool(name="cwp", bufs=1) as cwp,
            tc.tile_pool(name="xp", bufs=1) as xp,
            tc.tile_pool(name="tsb", bufs=1) as tsb,
            tc.tile_pool(name="stats", bufs=1) as stats,
            tc.tile_pool(name="pp", bufs=4, space="PSUM") as pp,
            tc.tile_pool(name="ppt", bufs=1, space="PSUM") as ppt,
            tc.tile_pool(name="ppf", bufs=1, space="PSUM") as ppf,
        ):
            # DMAs first: centers window, then x chunks in region order
            cwt = cwp.tile([P, KT2, 2, wc], _F8)
            nc.sync.dma_start(out=cwt[:, :, :, :], in_=cw[:, :, :, :])
            xt = xp.tile([P, nchunk, KT2, 2, ck], _F8)
            for q in range(nchunk):
                nc.sync.dma_start(out=xt[:, q, :, :, :], in_=xT[q, :, :, :, :])

            ones = const.tile([P, 1], _F32)
            nc.vector.memset(ones[:, :], 1.0)
            # [16,16] fp32 identity for PE transposes, built on device
            colix = const.tile([K, K], _F32)
            nc.gpsimd.iota(
                colix[:, :], pattern=[[1, K]], channel_multiplier=0,
                allow_small_or_imprecise_dtypes=True,
            )
            rowix = const.tile([K, 1], _F32)
            nc.gpsimd.iota(
                rowix[:, :], pattern=[[0, 1]], channel_multiplier=1,
                allow_small_or_imprecise_dtypes=True,
            )
            ident = const.tile([K, K], _F32)
            nc.vector.tensor_scalar(
                out=ident[:, :], in0=colix[:, :], scalar1=rowix[:, :],
                scalar2=None, op0=mybir.AluOpType.is_equal,
            )

            s_sb = tsb.tile([K, nreg, G], _F32)
            psT = ppt.tile([P, nreg, K], _F32)
            for r in range(nreg):
                q, rr = r // 4, r % 4
                ps = pp.tile([K, G], _F32, tag="ps", name=f"ps{r}")
                for j in range(KT2):
                    nc.tensor.matmul(
                        ps[:, :],
                        lhsT=cwt[:, j, :, K * r : K * (r + 1)],
                        rhs=xt[:, q, j, :, rr * G : (rr + 1) * G],
                        start=(j == 0),
                        stop=(j == KT2 - 1),
                        perf_mode=_DR,
                    )
                # PSUM -> SBUF, alternating engines so copies keep up
                if r % 2 == 0:
                    nc.vector.tensor_copy(out=s_sb[:, r, :], in_=ps[:, :])
                else:
                    nc.scalar.activation(
                        out=s_sb[:, r, :], in_=ps[:, :], func=_ACT.Copy,
                    )
                nc.tensor.transpose(psT[:, r, :], s_sb[:, r, :], ident[:, :])

            # batched stats on [P, nreg, K] -> [P, nreg], spread over engines
            lam = SCALE * SCALE
            ssum = stats.tile([P, nreg], _F32)
            nc.vector.tensor_reduce(
                out=ssum[:, :], in_=psT[:, :, :], axis=_AX, op=_ADD)
            mx = stats.tile([P, nreg], _F32)
            nc.vector.tensor_reduce(
                out=mx[:, :], in_=psT[:, :, :], axis=_AX, op=_MAX)
            # s^2 at true scale via Square((1/lam) * s~)
            sq = stats.tile([P, nreg, K], _F32)
            nc.scalar.activation(
                out=sq[:, :, :], in_=psT[:, :, :], func=_ACT.Square,
                scale=1.0 / lam,
            )
            qsum = stats.tile([P, nreg], _F32)
            nc.vector.tensor_reduce(
                out=qsum[:, :], in_=sq[:, :, :], axis=_AX, op=_ADD,
            )
            # epilogue on [96, ngrp, 8]:
            #   S = K - ssum/lam; Q = K - 2*ssum/lam + qsum; m = 1 - mx/lam
            #   rowloss = S - m + (2*m^2 - Q) / S
            md = stats.tile([P, nreg], _F32)
            nc.scalar.activation(
                out=md[:, :], in_=mx[:, :],
                func=_ACT.Copy, bias=1.0, scale=-1.0 / lam,
            )
            num = stats.tile([P, nreg], _F32)   # 2*m^2 = (sqrt(2)*m)^2
            nc.scalar.activation(
                out=num[:, :], in_=md[:, :], func=_ACT.Square,
                scale=1.41421356237,
            )
            sd = stats.tile([P, nreg], _F32)
            nc.vector.tensor_scalar(
                out=sd[:, :], in0=ssum[:, :], scalar1=-1.0 / lam,
                scalar2=float(K), op0=_MULT, op1=_ADD,
            )
            t = stats.tile([P, nreg], _F32)
            nc.vector.tensor_scalar(
                out=t[:, :], in0=ssum[:, :], scalar1=-2.0 / lam,
                scalar2=float(K), op0=_MULT, op1=_ADD,
            )
            qd = stats.tile([P, nreg], _F32)
            nc.vector.tensor_tensor(
                out=qd[:, :], in0=t[:, :], in1=qsum[:, :], op=_ADD
            )
            rs = stats.tile([P, nreg], _F32)
            nc.vector.reciprocal(rs[:, :], sd[:, :])
            num2 = stats.tile([P, nreg], _F32)
            nc.vector.tensor_tensor(
                out=num2[:, :], in0=num[:, :], in1=qd[:, :], op=_SUB
            )
            frac = stats.tile([P, nreg], _F32)
            nc.vector.tensor_tensor(
                out=frac[:, :], in0=num2[:, :], in1=rs[:, :], op=_MULT
            )
            base = stats.tile([P, nreg], _F32)
            nc.vector.tensor_tensor(
                out=base[:, :], in0=sd[:, :], in1=md[:, :], op=_SUB
            )
            rloss = stats.tile([P, nreg], _F32)
            nc.vector.tensor_tensor(
                out=rloss[:, :], in0=base[:, :], in1=frac[:, :], op=_ADD
            )
            rowsum = stats.tile([P, 1], _F32)
            nc.vector.tensor_reduce(
                out=rowsum[:, :],
                in_=rloss[:, :],
                axis=_AX, op=_ADD,
            )
            # cross-partition sum via ones-matmul -> single 4B output packet
            psc = ppf.tile([1, 1], _F32)
            nc.tensor.matmul(
                psc[:, :], lhsT=rowsum[:, :], rhs=ones[:, :], start=True, stop=True
            )
            outsb = stats.tile([1, 1], _F32)
            nc.vector.tensor_copy(out=outsb[:, :], in_=psc[:, :])
            nc.sync.dma_start(out=out[:, :], in_=outsb[:, :])

    nc.finalize()
    return nc


def _prep_inputs(x, labels, centers):
    """Host-side regioning/layout prep. Returns (in_maps, cfg)."""
    labels = np.asarray(labels).astype(np.int64)
    x = np.asarray(x, dtype=np.float32)
    centers = np.asarray(centers, dtype=np.float32)
    nclass = int(centers.shape[0])

    order = np.argsort(labels, kind="stable")
    ls = labels[order]
    starts = np.searchsorted(ls, np.arange(nclass))
    ends = np.searchsorted(ls, np.arange(nclass), side="right")

    # region list: (class, row_indices up to G)
    regions = []
    for c in range(nclass):
        rows = order[starts[c]:ends[c]]
        for o in range(0, len(rows), G):
            regions.append((c, rows[o:o + G]))

    nreg = -(-len(regions) // NCORES)          # per-core regions
    nreg = -(-nreg // 4) * 4                   # pad to chunk multiple
    total = nreg * NCORES
    regions += [(0, np.empty(0, dtype=np.int64))] * (total - len(regions))

    cols = nreg * G
    nchunk = nreg // 4
    ck = 4 * G
    f8 = ml_dtypes.float8_e4m3

    # centers, pre-transposed [D, C*K] and scaled
    centersT = (centers.reshape(nclass * K, D).T * SCALE).astype(f8)  # [D, C*K]
    zc = np.zeros((D, K), dtype=f8)

    in_maps = []
    nval = 0
    for i in range(NCORES):
        regs = regions[i * nreg:(i + 1) * nreg]
        xs = np.zeros((D, cols), dtype=np.float32)
        cwin = np.empty((D, K * nreg), dtype=f8)
        for r, (c, rows) in enumerate(regs):
            if len(rows):
                xs[:, r * G:r * G + len(rows)] = x[rows].T
                cwin[:, K * r:K * (r + 1)] = centersT[:, K * c:K * (c + 1)]
                nval += len(rows)
            else:
                cwin[:, K * r:K * (r + 1)] = zc
        xq = (xs * SCALE).astype(f8)
        # x: [nchunk, P, KT2, 2, ck] -- partition p of dktile-pair (j,h) is
        # feature dim 128*(2j+h)+p; chunk-major for contiguous DMA
        xdev = np.ascontiguousarray(
            xq.reshape(KT2, 2, P, nchunk, ck).transpose(3, 2, 0, 1, 4)
        )
        cwdev = np.ascontiguousarray(
            cwin.reshape(KT2, 2, P, K * nreg).transpose(2, 0, 1, 3)
        )
        in_maps.append({"xT": xdev, "cw": cwdev})
    # every (partition, slot) entry not holding a real row contributes
    # exactly PAD_LOSS: 8 slots/group/partition, half of them zero-slots
    npad = NCORES * nreg * G - nval
    return in_maps, (nreg, nchunk, npad)


def kernel(x, labels, centers):
    in_maps, (nreg, nchunk, npad) = _prep_inputs(x, labels, centers)
    nc = _build_program((nreg, nchunk))
    res = run_bass_kernel_spmd(nc, in_maps, core_ids=list(range(NCORES)))
    total = sum(float(r["out"].astype(np.float64).sum()) for r in res.results)
    return np.float32((total - npad * PAD_LOSS) / B)
